# revision 15
# baseline (speedup 1.0000x reference)
"""Trainium2 Bass kernel for ClassicalSelfAttention.

Math (per batch b):
    q = (x @ w_q.T) @ R ; k = (x @ w_k.T) @ Ent ; v = x @ w_v.T
    per head h: out_h = softmax(q_h @ k_h.T / sqrt(64)) @ v_h
    out[b, s, h*64+d]

Sharding: 8 cores, core i handles batch b = i // 4 and the adjacent head
pair m = i % 4 (global heads 2m, 2m+1 -> output columns 128m..128m+128).
Weights are replicated (column/row-sliced per head pair on the host); no
inter-core communication.

Per-core device plan (S = 4096, E = 512, D = 64, 2 heads):
  - host passes x^T (contraction dim E on partitions) so no on-device
    transpose of the 8 MB activation tensor is needed.
  - combined projection weights Wq_comb = w_q.T @ rot_cols (and w_k/ent)
    are computed on PE (tiny), so q/k projections are single matmuls.
  - Q^T, K^T stacked [128 = 2 heads x 64, S] in SBUF; V' = [V | ones]
    per head ([S, 65]) so the softmax denominator falls out of the PV
    matmul's extra column.
  - scores^T[k, q] = (K^T tile).T @ Q^T: contraction is D=64, so the two
    heads are packed onto disjoint PE row-groups (partitions 0-63 /
    64-127) and run concurrently.
  - exp via ScalarE directly PSUM->SBUF with scale=1/8 folded into the
    activation's free affine. No max subtraction: scores are ~N(0,1)
    (max |s| < ~8), safely inside fp32 exp range, matching softmax to
    float rounding.
  - PV: out'^T[65, q] = V'.T @ exp_sT accumulated over the 32 k-chunks in
    one PSUM bank per head; row 64 is the denominator. PE-transpose to
    natural layout, scale by reciprocal denominator, single DMA out.
"""

import sys

if "/opt/trn_rl_repo" not in sys.path:
    sys.path.insert(0, "/opt/trn_rl_repo")

import numpy as np

import concourse.bass as bass  # noqa: F401  (engine namespaces live on nc)
import concourse.mybir as mybir
import concourse.tile as tile
from concourse import bacc
from concourse.bass_utils import run_bass_kernel_spmd
from concourse.masks import make_identity

F32 = mybir.dt.float32
EXPF = mybir.ActivationFunctionType.Exp

E = 512
D = 64
PAIR = 128  # 2 heads x 64 dims per core
N_CORES = 8


def build_attention_nc(S=4096, lag=2, pv_dt="float16", mm_dt="float32r"):
    """Build the single-core Bass program (SPMD: every core runs this)."""
    EC = E // 128  # e-chunks (contraction over E)
    ST = S // 128  # s-tiles == k-tiles
    QB = S // 512  # query blocks (also projection blocks)
    EDT = getattr(mybir.dt, pv_dt) if isinstance(pv_dt, str) else pv_dt
    MMDT = getattr(mybir.dt, mm_dt) if isinstance(mm_dt, str) else mm_dt
    NSLOT = 3  # score slots (one head x k-tile each) per PSUM tile
    LAGS = 2 * lag  # PV lag in slots

    nc = bacc.Bacc("TRN2", target_bir_lowering=False, debug=False)

    xT_d = nc.dram_tensor("xT", [E, S], MMDT, kind="ExternalInput")
    wq_d = nc.dram_tensor("w_q", [E, E], MMDT, kind="ExternalInput")
    wk_d = nc.dram_tensor("w_k", [E, E], MMDT, kind="ExternalInput")
    rot_d = nc.dram_tensor("rot_cols", [E, PAIR], MMDT, kind="ExternalInput")
    ent_d = nc.dram_tensor("ent_cols", [E, PAIR], MMDT, kind="ExternalInput")
    wvT_d = nc.dram_tensor("wvT_cols", [E, PAIR], MMDT, kind="ExternalInput")
    out_d = nc.dram_tensor("out", [S, PAIR], F32, kind="ExternalOutput")

    with tile.TileContext(nc) as tc:
        with tc.tile_pool(name="persist", bufs=1) as PST:
            xT_sb = PST.tile([128, EC, S], MMDT)
            # per-block projection outputs so the main loop can start as
            # soon as block 0 is ready; remaining blocks interleave into
            # the loop's spare PE cycles.
            kTb = [PST.tile([128, 512], MMDT, name=f"kT_{b}") for b in range(QB)]
            qTb = [PST.tile([128, 512], MMDT, name=f"qT_{b}") for b in range(QB)]
            # V' per k-chunk: [V_h0 (64) | 1 | V_h1 (64) | 1]
            vb = [PST.tile([128, 4, 130], EDT, name=f"v_{b}") for b in range(QB)]
            out_sb = PST.tile([128, ST, PAIR], F32)
            ident = PST.tile([128, 128], F32)
            wqc_sb = PST.tile([128, EC, PAIR], MMDT)
            wkc_sb = PST.tile([128, EC, PAIR], MMDT)
            wvT_sb = PST.tile([128, EC, PAIR], MMDT)

            make_identity(nc, ident[:])
            # memset can't target fp32r/fp16; stage in fp32 and convert-copy
            ones_f32 = PST.tile([128, 4], F32)
            nc.vector.memset(ones_f32[:], 1.0)

            # ------------- weight DMAs + combined weights ---------------
            with (
                tc.tile_pool(name="wload", bufs=1) as WL,
                tc.tile_pool(name="comb_ps", bufs=2, space="PSUM") as CPS,
            ):
                wq_sb = WL.tile([128, EC, E], MMDT)
                wk_sb = WL.tile([128, EC, E], MMDT)
                rot_sb = WL.tile([128, EC, PAIR], MMDT)
                ent_sb = WL.tile([128, EC, PAIR], MMDT)
                for c in range(EC):
                    sl = slice(128 * c, 128 * (c + 1))
                    nc.sync.dma_start(out=wk_sb[:, c, :], in_=wk_d[sl, :])
                    nc.sync.dma_start(out=ent_sb[:, c, :], in_=ent_d[sl, :])
                    nc.sync.dma_start(out=wvT_sb[:, c, :], in_=wvT_d[sl, :])
                    nc.sync.dma_start(out=wq_sb[:, c, :], in_=wq_d[sl, :])
                    nc.sync.dma_start(out=rot_sb[:, c, :], in_=rot_d[sl, :])
                # x^T streamed per (block, chunk), block-major, so block b
                # projections can start before the whole tensor lands
                for b in range(QB):
                    bs = slice(512 * b, 512 * (b + 1))
                    for c in range(EC):
                        sl = slice(128 * c, 128 * (c + 1))
                        nc.sync.dma_start(out=xT_sb[:, c, bs], in_=xT_d[sl, bs])

                # Wk_comb first: K^T block 0 is the loop's first dependency
                for wsb, msb, dst in ((wk_sb, ent_sb, wkc_sb), (wq_sb, rot_sb, wqc_sb)):
                    for co in range(EC):
                        ps = CPS.tile([128, PAIR], F32, tag="comb_ps", name=f"cps_{co}")
                        for ct in range(EC):
                            nc.tensor.matmul(
                                ps[:],
                                lhsT=wsb[:, ct, 128 * co : 128 * (co + 1)],
                                rhs=msb[:, ct, :],
                                start=(ct == 0),
                                stop=(ct == EC - 1),
                            )
                        nc.vector.tensor_copy(dst[:, co, :], ps[:])

            # ---------------- attention main loop -----------------------
            with (
                tc.tile_pool(name="sc_ps", bufs=2, space="PSUM") as SC,
                tc.tile_pool(name="pv_ps", bufs=2, space="PSUM") as PVP,
                tc.tile_pool(name="exp_sb", bufs=4) as EX,
                tc.tile_pool(name="nrm_sb", bufs=2) as NRM,
            ):
                # projection emitters; psum borrowed from the score pool so
                # they can interleave with the loop without extra banks
                def emit_kqT(b, wc, dst, kind):
                    ps = SC.tile([128, NSLOT, 512], F32, tag="sc", name=f"{kind}ps_{b}")
                    bs = slice(512 * b, 512 * (b + 1))
                    for c in range(EC):
                        nc.tensor.matmul(
                            ps[:, 0, :],
                            lhsT=wc[:, c, :],
                            rhs=xT_sb[:, c, bs],
                            start=(c == 0),
                            stop=(c == EC - 1),
                        )
                    nc.vector.tensor_copy(dst[:], ps[:, 0, :])

                def emit_v(b):
                    ps = SC.tile([128, NSLOT, 512], F32, tag="sc", name=f"vps_{b}")
                    view = ps[:, 0, :].rearrange("p (j n) -> p j n", j=4)
                    for jj in range(4):
                        j = 4 * b + jj
                        for c in range(EC):
                            nc.tensor.matmul(
                                view[:, jj, :],
                                lhsT=xT_sb[:, c, 128 * j : 128 * (j + 1)],
                                rhs=wvT_sb[:, c, :],
                                start=(c == 0),
                                stop=(c == EC - 1),
                            )
                    nc.vector.tensor_copy(vb[b][:, :, 0:64], view[:, :, 0:64])
                    nc.vector.tensor_copy(vb[b][:, :, 65:129], view[:, :, 64:128])
                    nc.vector.tensor_copy(vb[b][:, :, 64:65], ones_f32[:])
                    nc.vector.tensor_copy(vb[b][:, :, 129:130], ones_f32[:])

                # minimal pre-loop: block 0 (+1 block of lookahead)
                emit_kqT(0, wkc_sb, kTb[0], "k")
                emit_kqT(0, wqc_sb, qTb[0], "q")
                emit_v(0)

                # remaining blocks fill qb0's spare PE cycles, always
                # ahead of first use (kT_b / v_b needed from kt = 4b)
                proj_sched = {}
                units = []
                for b in range(1, QB):
                    units.append(("k", b))
                    units.append(("v", b))
                for i, u in enumerate(units):
                    proj_sched.setdefault(1 + 2 * i, []).append(u)

                pending_norm = []

                def emit_norm():
                    # transpose + reciprocal-scale for the PREVIOUS q-block
                    # (deferred so it doesn't sit between two q-blocks on PE)
                    while pending_norm:
                        nqb, pvS_pair = pending_norm.pop(0)
                        for h in range(2):
                            pvS = pvS_pair[h]
                            tr = SC.tile([128, NSLOT, 512], F32, tag="sc", name=f"tr_{nqb}_{h}")
                            trv = tr[:, 0, :].rearrange("p (j n) -> p j n", j=4)[:, :, 0:65]
                            for c4 in range(4):
                                nc.tensor.transpose(
                                    trv[:, c4, :],
                                    pvS[:, 128 * c4 : 128 * (c4 + 1)],
                                    ident[0:65, 0:65],
                                )
                            rec = NRM.tile([128, 4], F32, tag="rec")
                            nc.vector.reciprocal(rec[:], trv[:, :, 64])
                            for c4 in range(4):
                                j = 4 * nqb + c4
                                nc.vector.tensor_scalar_mul(
                                    out_sb[:, j, 64 * h : 64 * (h + 1)],
                                    trv[:, c4, 0:64],
                                    rec[:, c4 : c4 + 1],
                                )
                        nc.sync.dma_start(
                            out=out_d[512 * nqb : 512 * (nqb + 1), :].rearrange(
                                "(j p) c -> p j c", p=128
                            ),
                            in_=out_sb[:, 4 * nqb : 4 * (nqb + 1), :],
                        )

                for qb in range(QB):
                    pv = [
                        PVP.tile([128, 512], F32, tag="pv", name=f"pv_h0_{qb}"),
                        PVP.tile([128, 512], F32, tag="pv", name=f"pv_h1_{qb}"),
                    ]
                    slot_et = [None] * (2 * ST)  # slot -> (exp tile, pos)
                    state = {"sc": None, "et": None, "acted": -1, "pv_next": 0}

                    def emit_pv(s, pv=pv, slot_et=slot_et):
                        kt, h = divmod(s, 2)
                        et, pos = slot_et[s]
                        nc.tensor.matmul(
                            pv[h][0:65, :],
                            lhsT=vb[kt // 4][:, kt % 4, 65 * h : 65 * h + 65],
                            rhs=et[:, pos, :],
                            start=(kt == 0),
                            stop=(kt == ST - 1),
                        )

                    def drain_pv(upto, state=state):
                        while state["pv_next"] <= upto:
                            emit_pv(state["pv_next"])
                            state["pv_next"] += 1

                    for kt in range(ST):
                        if qb == 0:
                            for kind, b in proj_sched.get(kt, ()):
                                if kind == "k":
                                    emit_kqT(b, wkc_sb, kTb[b], "k")
                                else:
                                    emit_v(b)
                        if kt == 2 and qb > 0:
                            emit_norm()
                        for h in range(2):
                            s = 2 * kt + h
                            pos = s % NSLOT
                            if pos == 0:
                                state["sc"] = SC.tile(
                                    [128, NSLOT, 512], F32, tag="sc", name=f"sc_{qb}_{s}"
                                )
                                state["et"] = EX.tile(
                                    [128, NSLOT, 512], EDT, tag="et", name=f"et_{qb}_{s}"
                                )
                            nc.tensor.matmul(
                                state["sc"][:, pos, :],
                                lhsT=kTb[kt // 4][64 * h : 64 * (h + 1), 128 * (kt % 4) : 128 * (kt % 4 + 1)],
                                rhs=qTb[qb][64 * h : 64 * (h + 1), :],
                                start=True,
                                stop=True,
                            )
                            slot_et[s] = (state["et"], pos)
                            if pos == NSLOT - 1:
                                nc.scalar.activation(
                                    state["et"][:], state["sc"][:], EXPF, scale=0.125
                                )
                                state["acted"] = s
                                drain_pv(state["acted"] - LAGS)
                    # flush partial tile + remaining PV
                    last = 2 * ST - 1
                    if state["acted"] < last:
                        pos = last % NSLOT
                        nc.scalar.activation(
                            state["et"][:, : pos + 1, :],
                            state["sc"][:, : pos + 1, :],
                            EXPF,
                            scale=0.125,
                        )
                    drain_pv(last)

                    # evacuate PV psum now (frees the pv slots for the next
                    # q-block); the transpose/normalize is deferred into the
                    # next q-block's early iterations
                    pvS_pair = []
                    for h in range(2):
                        pvS = NRM.tile([65, 512], F32, tag="pvS", name=f"pvS_{qb}_{h}")
                        nc.vector.tensor_copy(pvS[:], pv[h][0:65, :])
                        pvS_pair.append(pvS)
                    pending_norm.append((qb, pvS_pair))
                    if qb + 1 < QB:
                        emit_kqT(qb + 1, wqc_sb, qTb[qb + 1], "q")
                emit_norm()

    nc.compile()
    return nc


_NC_CACHE = {}

BUILD_OPTS = {"lag": 2, "pv_dt": "float16", "mm_dt": "float32r"}


def _get_nc(S=4096):
    key = (S, tuple(sorted(BUILD_OPTS.items())))
    if key not in _NC_CACHE:
        _NC_CACHE[key] = build_attention_nc(S=S, **BUILD_OPTS)
    return _NC_CACHE[key]


def _round_fp32r(a):
    """Round fp32 -> fp32r (11 mantissa bits, round-to-nearest-even)."""
    u = np.ascontiguousarray(a, dtype=np.float32).view(np.uint32).copy()
    u += np.uint32(0x7FF) + ((u >> np.uint32(12)) & np.uint32(1))
    u &= np.uint32(0xFFFFF000)
    return u.view(np.float32)


def _make_in_maps(rotation_params, entangle_params, inputs, w_q, w_k, w_v):
    B, S, E_ = inputs.shape
    assert E_ == E and B * 4 == N_CORES
    if BUILD_OPTS.get("mm_dt", "float32") == "float32r":
        f32 = lambda a: _round_fp32r(np.asarray(a, dtype=np.float32))
    else:
        f32 = lambda a: np.ascontiguousarray(np.asarray(a, dtype=np.float32))
    xTs = [f32(np.asarray(inputs[b]).T) for b in range(B)]
    w_q = f32(w_q)
    w_k = f32(w_k)
    rotation_params = np.asarray(rotation_params)
    entangle_params = np.asarray(entangle_params)
    w_v = np.asarray(w_v)
    in_maps = []
    for core in range(N_CORES):
        b, m = divmod(core, 4)
        cols = slice(PAIR * m, PAIR * (m + 1))
        in_maps.append(
            {
                "xT": xTs[b],
                "w_q": w_q,
                "w_k": w_k,
                "rot_cols": f32(rotation_params[:, cols]),
                "ent_cols": f32(entangle_params[:, cols]),
                "wvT_cols": f32(w_v[cols, :].T),
            }
        )
    return in_maps


def run(rotation_params, entangle_params, inputs, w_q, w_k, w_v, trace=False):
    """Run on the 8 NeuronCores; returns (output, BassKernelResults)."""
    inputs = np.asarray(inputs)
    B, S, E_ = inputs.shape
    nc = _get_nc(S)
    in_maps = _make_in_maps(rotation_params, entangle_params, inputs, w_q, w_k, w_v)
    res = run_bass_kernel_spmd(nc, in_maps, list(range(N_CORES)), trace=trace)
    out = np.empty((B, S, E_), dtype=np.float32)
    for core in range(N_CORES):
        b, m = divmod(core, 4)
        out[b, :, PAIR * m : PAIR * (m + 1)] = res.results[core]["out"]
    return out, res


def kernel(rotation_params, entangle_params, inputs, w_q, w_k, w_v):
    out, _ = run(rotation_params, entangle_params, inputs, w_q, w_k, w_v)
    return out


# revision 16
# speedup vs baseline: 1.1775x; 1.1775x over previous
"""Trainium2 Bass kernel for ClassicalSelfAttention.

Math (per batch b):
    q = (x @ w_q.T) @ R ; k = (x @ w_k.T) @ Ent ; v = x @ w_v.T
    per head h: out_h = softmax(q_h @ k_h.T / sqrt(64)) @ v_h
    out[b, s, h*64+d]

Sharding: 8 cores, core i handles batch b = i // 4 and the adjacent head
pair m = i % 4 (global heads 2m, 2m+1 -> output columns 128m..128m+128).
Weights are replicated (column/row-sliced per head pair on the host); no
inter-core communication.

Per-core device plan (S = 4096, E = 512, D = 64, 2 heads):
  - host passes x^T (contraction dim E on partitions) so no on-device
    transpose of the 8 MB activation tensor is needed.
  - combined projection weights Wq_comb = w_q.T @ rot_cols (and w_k/ent)
    are computed on PE (tiny), so q/k projections are single matmuls.
  - Q^T, K^T stacked [128 = 2 heads x 64, S] in SBUF; V' = [V | ones]
    per head ([S, 65]) so the softmax denominator falls out of the PV
    matmul's extra column.
  - scores^T[k, q] = (K^T tile).T @ Q^T: contraction is D=64, so the two
    heads are packed onto disjoint PE row-groups (partitions 0-63 /
    64-127) and run concurrently.
  - exp via ScalarE directly PSUM->SBUF with scale=1/8 folded into the
    activation's free affine. No max subtraction: scores are ~N(0,1)
    (max |s| < ~8), safely inside fp32 exp range, matching softmax to
    float rounding.
  - PV: out'^T[65, q] = V'.T @ exp_sT accumulated over the 32 k-chunks in
    one PSUM bank per head; row 64 is the denominator. PE-transpose to
    natural layout, scale by reciprocal denominator, single DMA out.
"""

import sys

if "/opt/trn_rl_repo" not in sys.path:
    sys.path.insert(0, "/opt/trn_rl_repo")

import numpy as np

import concourse.bass as bass  # noqa: F401  (engine namespaces live on nc)
import concourse.mybir as mybir
import concourse.tile as tile
from concourse import bacc
from concourse.bass_utils import run_bass_kernel_spmd
from concourse.masks import make_identity

F32 = mybir.dt.float32
EXPF = mybir.ActivationFunctionType.Exp

E = 512
D = 64
PAIR = 128  # 2 heads x 64 dims per core
N_CORES = 8


def build_attention_nc(S=4096, lag=2, pv_dt="float16", mm_dt="float32r"):
    """Build the single-core Bass program (SPMD: every core runs this)."""
    EC = E // 128  # e-chunks (contraction over E)
    ST = S // 128  # s-tiles == k-tiles
    QB = S // 512  # query blocks (also projection blocks)
    EDT = getattr(mybir.dt, pv_dt) if isinstance(pv_dt, str) else pv_dt
    MMDT = getattr(mybir.dt, mm_dt) if isinstance(mm_dt, str) else mm_dt
    NSLOT = 3  # score slots (one head x k-tile each) per PSUM tile
    LAGS = 2 * lag  # PV lag in slots

    nc = bacc.Bacc("TRN2", target_bir_lowering=False, debug=False)

    xT_d = nc.dram_tensor("xT", [E, S], MMDT, kind="ExternalInput")
    wq_d = nc.dram_tensor("w_q", [E, E], MMDT, kind="ExternalInput")
    wk_d = nc.dram_tensor("w_k", [E, E], MMDT, kind="ExternalInput")
    rot_d = nc.dram_tensor("rot_cols", [E, PAIR], MMDT, kind="ExternalInput")
    ent_d = nc.dram_tensor("ent_cols", [E, PAIR], MMDT, kind="ExternalInput")
    wvT_d = nc.dram_tensor("wvT_cols", [E, PAIR], MMDT, kind="ExternalInput")
    out_d = nc.dram_tensor("out", [S, PAIR], F32, kind="ExternalOutput")

    with tile.TileContext(nc) as tc:
        with tc.tile_pool(name="persist", bufs=1) as PST:
            xT_sb = PST.tile([128, EC, S], MMDT)
            # per-block projection outputs so the main loop can start as
            # soon as block 0 is ready; remaining blocks interleave into
            # the loop's spare PE cycles.
            kTb = [PST.tile([128, 512], MMDT, name=f"kT_{b}") for b in range(QB)]
            qTb = [PST.tile([128, 512], MMDT, name=f"qT_{b}") for b in range(QB)]
            # V' per k-chunk: [V_h0 (64) | 1 | V_h1 (64) | 1]
            vb = [PST.tile([128, 4, 130], EDT, name=f"v_{b}") for b in range(QB)]
            out_sb = PST.tile([128, ST, PAIR], F32)
            ident = PST.tile([128, 128], F32)
            wqc_sb = PST.tile([128, EC, PAIR], MMDT)
            wkc_sb = PST.tile([128, EC, PAIR], MMDT)
            wvT_sb = PST.tile([128, EC, PAIR], MMDT)

            make_identity(nc, ident[:])
            # memset can't target fp32r/fp16; stage in fp32 and convert-copy
            ones_f32 = PST.tile([128, 4], F32)
            nc.vector.memset(ones_f32[:], 1.0)

            # ------------- weight DMAs + combined weights ---------------
            with (
                tc.tile_pool(name="wload", bufs=1) as WL,
                tc.tile_pool(name="comb_ps", bufs=2, space="PSUM") as CPS,
            ):
                wq_sb = WL.tile([128, EC, E], MMDT)
                wk_sb = WL.tile([128, EC, E], MMDT)
                rot_sb = WL.tile([128, EC, PAIR], MMDT)
                ent_sb = WL.tile([128, EC, PAIR], MMDT)
                for c in range(EC):
                    sl = slice(128 * c, 128 * (c + 1))
                    nc.sync.dma_start(out=wk_sb[:, c, :], in_=wk_d[sl, :])
                    nc.sync.dma_start(out=ent_sb[:, c, :], in_=ent_d[sl, :])
                    nc.sync.dma_start(out=wvT_sb[:, c, :], in_=wvT_d[sl, :])
                    nc.sync.dma_start(out=wq_sb[:, c, :], in_=wq_d[sl, :])
                    nc.sync.dma_start(out=rot_sb[:, c, :], in_=rot_d[sl, :])
                # x^T streamed per (block, chunk), block-major, so block b
                # projections can start before the whole tensor lands
                for b in range(QB):
                    bs = slice(512 * b, 512 * (b + 1))
                    for c in range(EC):
                        sl = slice(128 * c, 128 * (c + 1))
                        nc.sync.dma_start(out=xT_sb[:, c, bs], in_=xT_d[sl, bs])

                # Wk_comb first: K^T block 0 is the loop's first dependency
                for wsb, msb, dst in ((wk_sb, ent_sb, wkc_sb), (wq_sb, rot_sb, wqc_sb)):
                    for co in range(EC):
                        ps = CPS.tile([128, PAIR], F32, tag="comb_ps", name=f"cps_{co}")
                        for ct in range(EC):
                            nc.tensor.matmul(
                                ps[:],
                                lhsT=wsb[:, ct, 128 * co : 128 * (co + 1)],
                                rhs=msb[:, ct, :],
                                start=(ct == 0),
                                stop=(ct == EC - 1),
                            )
                        nc.vector.tensor_copy(dst[:, co, :], ps[:])

            # ---------------- attention main loop -----------------------
            with (
                tc.tile_pool(name="sc_ps", bufs=2, space="PSUM") as SC,
                tc.tile_pool(name="pv_ps", bufs=2, space="PSUM") as PVP,
                tc.tile_pool(name="exp_sb", bufs=4) as EX,
                tc.tile_pool(name="nrm_sb", bufs=2) as NRM,
            ):
                # projection emitters; psum borrowed from the score pool so
                # they can interleave with the loop without extra banks
                def emit_kqT(b, wc, dst, kind):
                    ps = SC.tile([128, NSLOT, 512], F32, tag="sc", name=f"{kind}ps_{b}")
                    bs = slice(512 * b, 512 * (b + 1))
                    for c in range(EC):
                        nc.tensor.matmul(
                            ps[:, 0, :],
                            lhsT=wc[:, c, :],
                            rhs=xT_sb[:, c, bs],
                            start=(c == 0),
                            stop=(c == EC - 1),
                        )
                    nc.vector.tensor_copy(dst[:], ps[:, 0, :])

                def emit_v(b):
                    ps = SC.tile([128, NSLOT, 512], F32, tag="sc", name=f"vps_{b}")
                    view = ps[:, 0, :].rearrange("p (j n) -> p j n", j=4)
                    for jj in range(4):
                        j = 4 * b + jj
                        for c in range(EC):
                            nc.tensor.matmul(
                                view[:, jj, :],
                                lhsT=xT_sb[:, c, 128 * j : 128 * (j + 1)],
                                rhs=wvT_sb[:, c, :],
                                start=(c == 0),
                                stop=(c == EC - 1),
                            )
                    nc.vector.tensor_copy(vb[b][:, :, 0:64], view[:, :, 0:64])
                    nc.vector.tensor_copy(vb[b][:, :, 65:129], view[:, :, 64:128])
                    nc.vector.tensor_copy(vb[b][:, :, 64:65], ones_f32[:])
                    nc.vector.tensor_copy(vb[b][:, :, 129:130], ones_f32[:])

                # minimal pre-loop: block 0 (+1 block of lookahead)
                emit_kqT(0, wkc_sb, kTb[0], "k")
                emit_kqT(0, wqc_sb, qTb[0], "q")
                emit_v(0)
                if QB > 1:
                    emit_kqT(1, wkc_sb, kTb[1], "k")
                    emit_v(1)

                # remaining blocks fill qb0's spare PE cycles: one unit
                # every 2 k-tiles, always ahead of first use (kt = 4b)
                proj_sched = {}
                units = []
                for b in range(2, QB):
                    units.append(("k", b))
                    units.append(("v", b))
                for i, u in enumerate(units):
                    proj_sched.setdefault(1 + 2 * i, []).append(u)

                for qb in range(QB):
                    pv = [
                        PVP.tile([128, 512], F32, tag="pv", name=f"pv_h0_{qb}"),
                        PVP.tile([128, 512], F32, tag="pv", name=f"pv_h1_{qb}"),
                    ]
                    slot_et = [None] * (2 * ST)  # slot -> (exp tile, pos)
                    state = {"sc": None, "et": None, "acted": -1, "pv_next": 0}

                    def emit_pv(s, pv=pv, slot_et=slot_et):
                        kt, h = divmod(s, 2)
                        et, pos = slot_et[s]
                        nc.tensor.matmul(
                            pv[h][0:65, :],
                            lhsT=vb[kt // 4][:, kt % 4, 65 * h : 65 * h + 65],
                            rhs=et[:, pos, :],
                            start=(kt == 0),
                            stop=(kt == ST - 1),
                        )

                    def drain_pv(upto, state=state):
                        while state["pv_next"] <= upto:
                            emit_pv(state["pv_next"])
                            state["pv_next"] += 1

                    for kt in range(ST):
                        if qb == 0:
                            for kind, b in proj_sched.get(kt, ()):
                                if kind == "k":
                                    emit_kqT(b, wkc_sb, kTb[b], "k")
                                else:
                                    emit_v(b)
                        if kt == 16 and qb + 1 < QB:
                            emit_kqT(qb + 1, wqc_sb, qTb[qb + 1], "q")
                        for h in range(2):
                            s = 2 * kt + h
                            pos = s % NSLOT
                            if pos == 0:
                                state["sc"] = SC.tile(
                                    [128, NSLOT, 512], F32, tag="sc", name=f"sc_{qb}_{s}"
                                )
                                state["et"] = EX.tile(
                                    [128, NSLOT, 512], EDT, tag="et", name=f"et_{qb}_{s}"
                                )
                            nc.tensor.matmul(
                                state["sc"][:, pos, :],
                                lhsT=kTb[kt // 4][64 * h : 64 * (h + 1), 128 * (kt % 4) : 128 * (kt % 4 + 1)],
                                rhs=qTb[qb][64 * h : 64 * (h + 1), :],
                                start=True,
                                stop=True,
                            )
                            slot_et[s] = (state["et"], pos)
                            if pos == NSLOT - 1:
                                nc.scalar.activation(
                                    state["et"][:], state["sc"][:], EXPF, scale=0.125
                                )
                                state["acted"] = s
                                drain_pv(state["acted"] - LAGS)
                    # flush partial tile + remaining PV
                    last = 2 * ST - 1
                    if state["acted"] < last:
                        pos = last % NSLOT
                        nc.scalar.activation(
                            state["et"][:, : pos + 1, :],
                            state["sc"][:, : pos + 1, :],
                            EXPF,
                            scale=0.125,
                        )
                    drain_pv(last)

                    # normalize + transpose to natural layout
                    for h in range(2):
                        pvS = NRM.tile([65, 512], F32, tag="pvS")
                        nc.vector.tensor_copy(pvS[:], pv[h][0:65, :])
                        tr = PVP.tile([128, 4, 65], F32, tag="pv", name=f"tr_{qb}_{h}")
                        for c4 in range(4):
                            nc.tensor.transpose(
                                tr[:, c4, :],
                                pvS[:, 128 * c4 : 128 * (c4 + 1)],
                                ident[0:65, 0:65],
                            )
                        rec = NRM.tile([128, 4], F32, tag="rec")
                        nc.vector.reciprocal(rec[:], tr[:, :, 64])
                        for c4 in range(4):
                            j = 4 * qb + c4
                            nc.vector.tensor_scalar_mul(
                                out_sb[:, j, 64 * h : 64 * (h + 1)],
                                tr[:, c4, 0:64],
                                rec[:, c4 : c4 + 1],
                            )
                    nc.sync.dma_start(
                        out=out_d[512 * qb : 512 * (qb + 1), :].rearrange(
                            "(j p) c -> p j c", p=128
                        ),
                        in_=out_sb[:, 4 * qb : 4 * (qb + 1), :],
                    )

    nc.compile()
    return nc


_NC_CACHE = {}

BUILD_OPTS = {"lag": 2, "pv_dt": "float16", "mm_dt": "float32r"}


def _get_nc(S=4096):
    key = (S, tuple(sorted(BUILD_OPTS.items())))
    if key not in _NC_CACHE:
        _NC_CACHE[key] = build_attention_nc(S=S, **BUILD_OPTS)
    return _NC_CACHE[key]


def _round_fp32r(a):
    """Round fp32 -> fp32r (11 mantissa bits, round-to-nearest-even)."""
    u = np.ascontiguousarray(a, dtype=np.float32).view(np.uint32).copy()
    u += np.uint32(0x7FF) + ((u >> np.uint32(12)) & np.uint32(1))
    u &= np.uint32(0xFFFFF000)
    return u.view(np.float32)


def _make_in_maps(rotation_params, entangle_params, inputs, w_q, w_k, w_v):
    B, S, E_ = inputs.shape
    assert E_ == E and B * 4 == N_CORES
    if BUILD_OPTS.get("mm_dt", "float32") == "float32r":
        f32 = lambda a: _round_fp32r(np.asarray(a, dtype=np.float32))
    else:
        f32 = lambda a: np.ascontiguousarray(np.asarray(a, dtype=np.float32))
    xTs = [f32(np.asarray(inputs[b]).T) for b in range(B)]
    w_q = f32(w_q)
    w_k = f32(w_k)
    rotation_params = np.asarray(rotation_params)
    entangle_params = np.asarray(entangle_params)
    w_v = np.asarray(w_v)
    in_maps = []
    for core in range(N_CORES):
        b, m = divmod(core, 4)
        cols = slice(PAIR * m, PAIR * (m + 1))
        in_maps.append(
            {
                "xT": xTs[b],
                "w_q": w_q,
                "w_k": w_k,
                "rot_cols": f32(rotation_params[:, cols]),
                "ent_cols": f32(entangle_params[:, cols]),
                "wvT_cols": f32(w_v[cols, :].T),
            }
        )
    return in_maps


def run(rotation_params, entangle_params, inputs, w_q, w_k, w_v, trace=False):
    """Run on the 8 NeuronCores; returns (output, BassKernelResults)."""
    inputs = np.asarray(inputs)
    B, S, E_ = inputs.shape
    nc = _get_nc(S)
    in_maps = _make_in_maps(rotation_params, entangle_params, inputs, w_q, w_k, w_v)
    res = run_bass_kernel_spmd(nc, in_maps, list(range(N_CORES)), trace=trace)
    out = np.empty((B, S, E_), dtype=np.float32)
    for core in range(N_CORES):
        b, m = divmod(core, 4)
        out[b, :, PAIR * m : PAIR * (m + 1)] = res.results[core]["out"]
    return out, res


def kernel(rotation_params, entangle_params, inputs, w_q, w_k, w_v):
    out, _ = run(rotation_params, entangle_params, inputs, w_q, w_k, w_v)
    return out
